# revision 61
# baseline (speedup 1.0000x reference)
"""Fast Feedforward (FFF) tree-routing kernel for Trainium2, 8 NeuronCores.

Problem: B=8192 tokens, d=4096, binary tree depth 12 (4095 nodes).
Per token, per level: logit = <x, w1s[node]>; y += gelu(logit) * w2s[node];
node = 2*node + 1 + (logit > 0).

Strategy (data-parallel over tokens, 1024 tokens/core, 8 tiles of 128):
- Levels 0-8 (511 nodes): dense logits L = x @ W1[0:511]^T via PE matmul
  (host-pretransposed xT and W1T tables, feature-major chunks).
  Routing = per-level select/compare ops on L (DVE); gelu via one native
  ACT Gelu_apprx_tanh op; y contribution via S^T-transpose matmuls
  @ W2[0:511].
- Levels 9-11: per 128-token tile, the gather idx tile (int16, 16-wrapped
  + stripe-replicated) is built by a tiny fp32 PE matmul; dma_gather
  fetches the 128 w1 rows (bf16) AND the 128 w2 rows (fp8e4, table
  host-scaled by 64 — halves that DMA) per level on the same idx, so the
  deep phase keeps the DMA device as busy as the DVE dot chain. The
  per-token dot runs as ONE fused DVE scalar_tensor_tensor with accum_out;
  gelu + the diag(gelu/64) tile are produced immediately (ACT + DVE), so
  stage 2 has no DVE dependencies. y folds the deep rows in with
  mixed-dtype (bf16 diag x fp8 row) PE matmuls.
- y accumulates in PSUM fp32 quarter slices, ACT copies into one bf16
  tile, stored with two half-row DMAs issued on the ACT queue (so the
  store never parks a sequencer on a foreign dependency).
- Pipeline: groups of [3,3,2] tiles, software-pipelined over 3 stages.
  Routing (s1) runs in the same step as its L matmuls, so each level's
  idx matmuls are ready when the PE reaches them; combine bursts of the
  output group interleave between deep levels to hide dot latency.
"""

import numpy as np
import ml_dtypes

import concourse.bacc as bacc
import concourse.bass as bass
import concourse.mybir as mybir
import concourse.tile as tile
from concourse.bass import ts
from concourse.masks import make_identity

P = 128
IN = 4096
OUT = 4096
DEPTH = 12
N_NODES = 2**DEPTH - 1          # 4095
N_CORES = 8
B = 8192
TOK = B // N_CORES              # 1024 tokens per core
NT = TOK // P                   # 8 tiles of 128 tokens
CH = IN // P                    # 32 feature chunks
SH_LV = 9                       # dense shallow levels 0..8
SH_NODES = 2**SH_LV - 1         # 511
SH_PAD = 512
SH_CH = SH_PAD // P             # 4 node chunks for shallow combine
DEEP_LV = list(range(SH_LV, DEPTH))   # [9, 10, 11]
NQ = 8                          # y feature quarters
QW = OUT // NQ                  # 512
BF = mybir.dt.bfloat16
F32 = mybir.dt.float32
FP8 = mybir.dt.float8e4
I16 = mybir.dt.int16
AF = mybir.ActivationFunctionType
OP = mybir.AluOpType

W2_SCALE = 64.0                 # host-side fp8 table scale
DOT_SPLIT = 4096                # cols of the deep dot done fused on DVE


def build_program(n_tiles=NT, num_devices=N_CORES, w2fp8=True,
                  dot_split=DOT_SPLIT, repeat=1):
    nc = bacc.Bacc("TRN2", target_bir_lowering=False, debug=False,
                   num_devices=num_devices, num_swdge_queues=4)
    # xT is host-prepped per-tile chunked: row (t*128+p) holds features
    # [p, 128+p, 256+p, ...] of the tile's 128 tokens -> each partition
    # reads one contiguous 8KB line per tile load.
    xT = nc.dram_tensor("xT", [n_tiles * P, CH * P], BF, kind="ExternalInput")
    x_tm = nc.dram_tensor("x", [n_tiles * P, IN], BF, kind="ExternalInput")
    w1t_sh = nc.dram_tensor("w1t_sh", [IN, SH_PAD], BF, kind="ExternalInput")
    w1s = nc.dram_tensor("w1s", [N_NODES, IN], BF, kind="ExternalInput")
    if w2fp8:
        w2d = nc.dram_tensor("w2s8", [N_NODES, IN], FP8, kind="ExternalInput")
    else:
        w2d = nc.dram_tensor("w2s", [N_NODES, IN], BF, kind="ExternalInput")
    w2sh = nc.dram_tensor("w2sh", [SH_PAD, OUT], BF, kind="ExternalInput")
    y = nc.dram_tensor("y", [n_tiles * P, OUT], BF, kind="ExternalOutput")
    wsel_d = nc.dram_tensor("wsel", [P, P], F32, kind="ExternalInput")
    m8_d = nc.dram_tensor("m8", [P, 8], F32, kind="ExternalInput")

    w1t_sh_r = w1t_sh.rearrange("(c p) n -> p c n", p=P)  # [128, 32, 512]
    w2_sh_r = w2sh.rearrange("(j p) f -> p j f", p=P)     # [128, 4, 4096]

    qn_counter = [0]

    def qn():
        q = qn_counter[0] % 4
        qn_counter[0] += 1
        return q

    gscale = 1.0 / W2_SCALE if w2fp8 else 1.0

    with tile.TileContext(nc) as tc:
        with (
            tc.tile_pool(name="singles", bufs=1) as singles,
            tc.tile_pool(name="xpool", bufs=3) as xpool,
            tc.tile_pool(name="xtokpool", bufs=3) as xtokp,
            tc.tile_pool(name="spool", bufs=3) as spool,
            tc.tile_pool(name="small", bufs=6) as small,
            tc.tile_pool(name="deep", bufs=9) as deep,
            tc.tile_pool(name="diagp", bufs=9) as diagp,
            tc.tile_pool(name="w1gpool", bufs=3) as w1gp,
            tc.tile_pool(name="idxsave", bufs=8) as idxsave,
            tc.tile_pool(name="ypool", bufs=2) as ypool,
            tc.tile_pool(name="lps", bufs=2, space="PSUM") as lps,
            tc.tile_pool(name="idxps", bufs=2, space="PSUM") as idxps,
            tc.tile_pool(name="stps", bufs=1, space="PSUM") as stps,
            tc.tile_pool(name="yps", bufs=2, space="PSUM") as yps,
        ):
            # --- persistent tables ---
            # w1t loads in 4 chunk-groups so the first L matmuls only wait
            # on the first 1MB, not the whole 4MB table
            w1t_sb = singles.tile([P, CH, SH_PAD], BF)
            for c4 in range(0, CH, 8):
                nc.scalar.dma_start(out=w1t_sb[:, c4:c4 + 8, :],
                                    in_=w1t_sh_r[:, c4:c4 + 8, :])
            w2sh_sb = singles.tile([P, SH_CH, OUT], BF)
            nc.scalar.dma_start(out=w2sh_sb[:], in_=w2_sh_r[:])
            ident = singles.tile([P, P], BF)
            make_identity(nc, ident[:])
            # identity pre-scaled by 1/W2_SCALE for the fp8 diag combine
            ident_s = singles.tile([P, P], BF, tag="ident_s")
            nc.vector.tensor_scalar(out=ident_s[:], in0=ident[:],
                                    scalar1=gscale, scalar2=None,
                                    op0=OP.mult)
            wsel = singles.tile([P, P], F32, tag="wsel")
            nc.scalar.dma_start(out=wsel[:], in_=wsel_d[:])
            m8 = singles.tile([P, 8], F32, tag="m8")
            nc.scalar.dma_start(out=m8[:], in_=m8_d[:])

            iota_f = singles.tile([P, SH_PAD], F32)
            nc.gpsimd.iota(iota_f[:], pattern=[[1, SH_PAD]], base=0,
                           channel_multiplier=0,
                           allow_small_or_imprecise_dtypes=True)

            state = {}

            def s0(t):
                # stage 0: x loads + dense shallow logits
                xt = xpool.tile([P, CH, P], BF, tag="xt")
                nc.sync.dma_start(
                    out=xt[:],
                    in_=xT[ts(t, P), :].rearrange("p (c b) -> p c b", c=CH))
                xtok = xtokp.tile([P, IN], BF, tag="xtok")
                nc.sync.dma_start(out=xtok[:], in_=x_tm[ts(t, P), :])
                l_ps = lps.tile([P, SH_PAD], F32)
                for c in range(CH):
                    nc.tensor.matmul(l_ps[:], lhsT=xt[:, c, :],
                                     rhs=w1t_sb[:, c, :],
                                     start=(c == 0), stop=(c == CH - 1))
                l_sb = spool.tile([P, SH_PAD], BF, tag="lsb")
                nc.scalar.copy(out=l_sb[:], in_=l_ps[:])
                state[t] = {"xtok": xtok, "l_sb": l_sb}

            def s1_shallow(t):
                # stage 1a: shallow routing + gelu over masked logits + S^T
                stt = state[t]
                l_sb = stt["l_sb"]
                ml = spool.tile([P, SH_PAD], BF, tag="ml")
                nc.vector.memset(ml[:], 0.0)
                node = small.tile([P, 1], F32, tag="node")
                nc.vector.memset(node[:], 0.0)
                for d in range(SH_LV):
                    lo, w = 2**d - 1, 2**d
                    logit = small.tile([P, 1], F32, tag="logit")
                    # ML[:, lo:lo+w] = (iota == node) * L ; accum -> logit
                    nc.vector.scalar_tensor_tensor(
                        out=ml[:, lo:lo + w],
                        in0=iota_f[:, lo:lo + w],
                        scalar=node[:, :1],
                        in1=l_sb[:, lo:lo + w],
                        op0=OP.is_equal, op1=OP.mult,
                        accum_out=logit[:, :1])
                    b1 = small.tile([P, 1], F32, tag="b1")
                    nc.vector.tensor_scalar(
                        out=b1[:], in0=logit[:], scalar1=0.0, scalar2=1.0,
                        op0=OP.is_gt, op1=OP.add)
                    nc.vector.scalar_tensor_tensor(
                        out=node[:], in0=node[:], scalar=2.0, in1=b1[:],
                        op0=OP.mult, op1=OP.add)

                # S = gelu(ML): one native ACT op in place; zeros stay zero
                gl = ml
                nc.scalar.activation(out=gl[:], in_=ml[:],
                                     func=AF.Gelu_apprx_tanh)
                st_ps = stps.tile([P, SH_CH, P], BF)
                for j in range(SH_CH):
                    nc.tensor.transpose(st_ps[:, j, :], gl[:, ts(j, P)],
                                        ident[:])
                st_sb = spool.tile([P, SH_CH, P], BF, tag="stsb")
                nc.scalar.copy(out=st_sb[:], in_=st_ps[:])

                stt["st_sb"] = st_sb
                stt["node"] = node
                stt["w2g_t"] = {}
                stt["diag_t"] = {}

            def deep_issue(t, d):
                # idx[p, cc] = node[16cc + p%16] via tiny fp32 matmul,
                # then launch the w1/w2 row gathers for this level
                stt = state[t]
                node = stt["node"]
                rhs8 = small.tile([P, 8], F32, tag="rhs8")
                nc.vector.tensor_scalar(out=rhs8[:], in0=m8[:],
                                        scalar1=node[:, :1],
                                        scalar2=None, op0=OP.mult)
                idx_ps = idxps.tile([P, 8], F32, tag="idxps")
                nc.tensor.matmul(idx_ps[:], lhsT=wsel[:], rhs=rhs8[:],
                                 start=True, stop=True)
                idx = idxsave.tile([P, P // 16], I16, tag="idx")
                nc.vector.tensor_copy(out=idx[:], in_=idx_ps[:])
                w1g = w1gp.tile([P, 1, IN], BF, tag="w1g")
                nc.gpsimd.dma_gather(
                    w1g[:], w1s[:, :], idx[:, :], P, P, IN,
                    transpose=False, queue_num=qn())
                # co-issue the (fp8) w2 row gather on the same idx so the
                # deep phase keeps the DMA device as busy as the DVE dots
                w2dt = FP8 if w2fp8 else BF
                w2g = deep.tile([P, 1, IN], w2dt, tag="w2g")
                nc.gpsimd.dma_gather(
                    w2g[:], w2d[:, :], idx[:, :], P, P, IN,
                    transpose=False, queue_num=qn())
                stt["w2g_t"][d] = w2g
                stt["w1g"] = w1g

            def deep_consume(t, d):
                # fused dot: ONE DVE scalar_tensor_tensor with accum_out
                stt = state[t]
                node, xtok, w1g = stt["node"], stt["xtok"], stt["w1g"]
                ds = dot_split
                la = small.tile([P, 1], F32, tag="la")
                nc.vector.scalar_tensor_tensor(
                    out=w1g[:, 0, 0:ds], in0=xtok[:, 0:ds], scalar=1.0,
                    in1=w1g[:, 0, 0:ds], op0=OP.bypass, op1=OP.mult,
                    accum_out=la[:, :1])
                logit = la
                g = idxsave.tile([P, 1], F32, tag="g")
                nc.scalar.activation(out=g[:], in_=logit[:],
                                     func=AF.Gelu_apprx_tanh)
                dg = diagp.tile([P, P], BF, tag="diag")
                nc.vector.tensor_scalar(
                    out=dg[:], in0=ident_s[:], scalar1=g[:, :1],
                    scalar2=None, op0=OP.mult)
                stt["diag_t"][d] = dg
                if d < DEPTH - 1:
                    b1 = small.tile([P, 1], F32, tag="b1")
                    nc.vector.tensor_scalar(
                        out=b1[:], in0=logit[:], scalar1=0.0,
                        scalar2=1.0, op0=OP.is_gt, op1=OP.add)
                    nc.vector.scalar_tensor_tensor(
                        out=node[:], in0=node[:], scalar=2.0,
                        in1=b1[:], op0=OP.mult, op1=OP.add)

            def s2(t):
                # stage 2: y combine + store (pure PE/ACT/DMA — no DVE deps)
                if t not in state:
                    return
                stt = state.pop(t)
                st_sb = stt["st_sb"]
                diag_t, w2g_t = stt["diag_t"], stt["w2g_t"]
                y_sb = ypool.tile([P, OUT], BF, tag="ysb")
                for q in range(NQ):
                    y_ps = yps.tile([P, QW], F32)
                    col0 = q * QW
                    first = True
                    for d in DEEP_LV:
                        nc.tensor.matmul(
                            y_ps[:],
                            lhsT=diag_t[d][:],
                            rhs=w2g_t[d][:, 0, col0:col0 + QW],
                            start=first, stop=False)
                        first = False
                    for j in range(SH_CH):
                        nc.tensor.matmul(
                            y_ps[:],
                            lhsT=st_sb[:, j, :],
                            rhs=w2sh_sb[:, j, col0:col0 + QW],
                            start=first, stop=(j == SH_CH - 1))
                        first = False
                    nc.scalar.copy(out=y_sb[:, col0:col0 + QW], in_=y_ps[:])
                    if q % 2 == 1:
                        lo = (q - 1) * QW
                        nc.scalar.dma_start(
                            out=y[ts(t, P), lo:lo + 2 * QW],
                            in_=y_sb[:, lo:lo + 2 * QW])

            # Software pipeline over tile groups. Emission order matters
            # because every engine sequencer is in-order:
            # - per deep level: ALL dots (DVE) first, then one combine
            #   burst (PE) that runs concurrently with them, then the next
            #   level's idx matmuls (PE) — which become ready just as the
            #   PE finishes the burst — then the DVE idx copies, so they
            #   never head-block the dot chain.
            # - first/last groups are small to shorten prologue/epilogue.
            sizes = [3, 3, 2]
            groups, pos = [], 0
            for s in sizes:
                groups.append(list(range(pos, min(pos + s, n_tiles))))
                pos += s
            groups = [g for g in groups if g]
            ng = len(groups)

            for _rep in range(repeat):
                for m in range(ng + 2):
                    g_cur = groups[m] if m < ng else []
                    g_deep = groups[m - 1] if 1 <= m <= ng else []
                    g_out = groups[m - 2] if 2 <= m else []
                    out_iter = list(g_out)

                    def emit_one_s2():
                        if out_iter:
                            s2(out_iter.pop(0))

                    # g_deep's routing already ran last step (s1 moved into
                    # the s0 step), so its idx9 matmuls are ready at step
                    # start; combine bursts of g_out interleave between
                    # deep levels to keep PE dense while dots grind on DVE.
                    if g_deep:
                        for a in g_deep:
                            deep_issue(a, DEEP_LV[0])
                        for d in DEEP_LV:
                            for a in g_deep:
                                deep_consume(a, d)
                                if d + 1 in DEEP_LV:
                                    deep_issue(a, d + 1)
                            emit_one_s2()
                    while out_iter:
                        emit_one_s2()
                    for a in g_cur:
                        s0(a)
                    for a in g_cur:
                        s1_shallow(a)

    nc.compile()
    return nc


_CACHED = {}


def _get_program(n_tiles=NT, num_devices=N_CORES, **kw):
    key = (n_tiles, num_devices, tuple(sorted(kw.items())))
    if key not in _CACHED:
        _CACHED[key] = build_program(n_tiles, num_devices, **kw)
    return _CACHED[key]


def idx_masks():
    i = np.arange(P)
    wsel = (i[:, None] % 16 == i[None, :] % 16).astype(np.float32)
    m8 = (i[:, None] // 16 == np.arange(8)[None, :]).astype(np.float32)
    return wsel, m8


def prep_inputs(input, w1s, w2s):
    """Host-side layout prep shared by all cores."""
    x = np.asarray(input)
    xr = x.reshape(B // P, P, CH, P).transpose(0, 3, 2, 1)
    xT = np.ascontiguousarray(xr.reshape(B, CH * P))
    w1 = np.asarray(w1s)
    w1t_sh = np.zeros((IN, SH_PAD), dtype=w1.dtype)
    w1t_sh[:, :SH_NODES] = w1[:SH_NODES].T
    w2 = np.asarray(w2s)
    w2sh = np.zeros((SH_PAD, OUT), dtype=w2.dtype)
    w2sh[:SH_NODES] = w2[:SH_NODES]
    w2f8 = (w2.astype(np.float32) * W2_SCALE).astype(ml_dtypes.float8_e4m3fn)
    return xT, np.ascontiguousarray(w1t_sh), w2sh, w2f8


def _run(input, w1s, w2s, **spmd_kwargs):
    from concourse.bass_utils import run_bass_kernel_spmd

    nc = _get_program()
    xT, w1t_sh, w2sh, w2f8 = prep_inputs(input, w1s, w2s)
    w1 = np.ascontiguousarray(np.asarray(w1s))
    wsel, m8 = idx_masks()
    in_maps = []
    for c in range(N_CORES):
        in_maps.append({
            "xT": np.ascontiguousarray(xT[c * TOK:(c + 1) * TOK, :]),
            "x": np.ascontiguousarray(np.asarray(input)[c * TOK:(c + 1) * TOK]),
            "w1t_sh": w1t_sh,
            "w1s": w1,
            "w2s8": w2f8,
            "w2sh": w2sh,
            "wsel": wsel,
            "m8": m8,
        })
    res = run_bass_kernel_spmd(nc, in_maps, core_ids=list(range(N_CORES)),
                               **spmd_kwargs)
    out = np.concatenate([res.results[c]["y"] for c in range(N_CORES)], axis=0)
    return out.astype(ml_dtypes.bfloat16), res


def kernel(input, w1s, w2s, depth):
    assert int(depth) == DEPTH
    out, _ = _run(input, w1s, w2s)
    return out
